# revision 49
# baseline (speedup 1.0000x reference)
"""ConvTasNet-style decoder kernel for Trainium2 (8 NeuronCores).

Computation (per batch m, channel c):
    s[n,k]    = mixture_w[n,k] * est_mask[c,n,k]          n=512, k=16000
    frames    = W @ s                                      [16, 16000]
    out[8q+r] = frames[r, q] + frames[r+8, q-1]            overlap-add, hop 8

Sharding: 8 cores = (m in 0..3) x (k-half in 0..1). Each core handles both
channels of one (m, k-half); mixture_w is read once per core.

The kernel is HBM-DMA-bound at fp16, so inputs ship as 8 bits/element:
  mixture_w -> int8 with a per-row scale s[n] = absmax(mw[n,:])/127
  est_mask  -> uint8 = round(mask*255), EXCEPT the chunks in FP16_CHUNKS
               which ship as fp16 (trades spare DMA bandwidth for vector-
               engine relief; the engines are the bottleneck once inputs
               are 8-bit)
Both quantization scales are folded into the per-core stationary weights
W'[n,l] = W[l,n] * s[n] * 2^12 / 255 (fp16; the 2^12 keeps W' out of the
fp16 subnormal range and is divided back out on the host).

On device, mixture_w is upconverted once to fp16 (Scalar/Vector engines),
and the mask multiply runs MIXED-dtype straight off the uint8 mask:
st = mask_u8 * mwf_f16 -> f16 on the Vector engine (1 elem/cycle) and
GpSimd engine, split by column ranges via a load-balancing waterfill.
fp16-mask chunks multiply at the DVE's 2 elem/cycle fp16 rate. No
separate mask dequantization pass exists; that is the main win over a
convert-then-multiply structure.

The overlap-add happens on the HOST: the device writes the raw frame
matrix (rows 0:16 of W', both channels) and the host does
out[8q+r] = frames[r,q] + frames[r+8,q-1]. Per 512-column PSUM group one
engine copy stages PSUM f32 -> SBUF fp16 (48 partitions at once: c0 in
PSUM partitions 0:16, c1 in 32:48, written by zero-padded stationaries
so every partition is matmul-initialized); the per-chunk output DMA is
issued on the engine that ran the chunk's last stage copy, so its data
wait never head-of-line-blocks another engine queue.

Scheduling notes (from the timeline cost model):
 - Matmul speed ramps with sustained PE activity (low->mid->full over
   3us, reset when the PE starves). Warmup matmuls plus small leading
   chunks get the PE to full clock before the main stream.
 - Engines execute their instruction streams in order, so the
   matmul/stage/output phase of chunk i is EMITTED one chunk behind the
   DMA/convert/multiply phase (software pipelining of the emission
   order); otherwise a stage op waiting on matmuls blocks the next
   chunk's multiplies queued behind it.
 - The DVE's column share of each multiply is emitted first (columns
   0:x) so the chunk's first PSUM group can start its matmuls while the
   GpSimd engine is still producing the tail columns.
"""

import sys

sys.path.insert(0, "/opt/trn_rl_repo")

import numpy as np

M, C, N, K, L = 4, 2, 512, 16000, 16
HOP = L // 2            # 8
KH = K // 2             # 8000 frames per core
QH = KH + 1             # 8001 output blocks per core per channel
TH = QH * HOP           # 64008 samples per core per channel
T = (K - 1) * HOP + L   # 128008 full output samples
SUB = 512               # PSUM accumulation group width (one bank of fp32)
NB = N // 128           # 4 contraction subtiles
SCALE_BITS = 12         # W' = W * s[n] * 2^SCALE_BITS / 255
CHUNKS = [512, 768, 1024, 1024, 1024, 1024, 1024, 1024, 576]  # sum=KH
FP16_CHUNKS = frozenset({0, 3, 5, 7})  # chunk indices whose mask ships as fp16
WARMUP_MM = 0           # PE warmup matmuls before the real stream
IN_BUFS = 4             # input-chunk buffer depth
ST_BUFS = 3             # product-tile depth (consumed one chunk later)
MIN_COLS = 64           # smallest column-split a balancer share may get
_CACHE = {}

# Engine cost model (ns/elem + fixed ns/op) calibrated against the
# timeline simulator.
_CONV_RATE = {"DVE": 0.5208, "ACT": 0.8333, "Pool": 1.3889}   # 8bit->fp16
_MIXED_RATE = {"DVE": 1.0417, "Pool": 1.9841}                 # u8*f16 mul
_F16MUL_RATE = {"DVE": 0.5208, "Pool": 1.9841}                # f16*f16 mul
_STAGE_RATE = {"DVE": 1.0417, "ACT": 0.8333}  # psum->sbuf (no Pool: GPSIMD cannot access PSUM)
_FIXED = {"DVE": 115, "ACT": 190, "Pool": 100}


def _build_nc():
    import concourse.tile as tile
    from concourse import bacc, mybir

    f32 = mybir.dt.float32
    f16 = mybir.dt.float16
    i8 = mybir.dt.int8
    u8 = mybir.dt.uint8

    kh8 = sum(ck for ci, ck in enumerate(CHUNKS) if ci not in FP16_CHUNKS)
    kh16 = KH - kh8

    nc = bacc.Bacc("TRN2", target_bir_lowering=False, debug=False, num_devices=8)
    mw_d = nc.dram_tensor("mw", [N, KH], i8, kind="ExternalInput")
    mask8_d = nc.dram_tensor("mask8", [C * N, kh8], u8, kind="ExternalInput")
    mask16_d = nc.dram_tensor("mask16", [C * N, kh16], f16, kind="ExternalInput")
    # W' transposed and pre-padded to a 32-partition PSUM layout on the
    # host: w0 = [W', 0], w1 = [0, W'] ([N, 32] each). Both channels'
    # matmuls then write PSUM partitions 0:32 (32-aligned base, contiguous),
    # so the stage copy and the output DMA use plain single-level partition
    # access patterns -- a two-level partition AP in a DMA reads garbage on
    # real hardware even though the simulator accepts it. Shipping the
    # padding also avoids an engine memset sharing a tile with a DMA-written
    # sub-region, which corrupts sporadically on hardware.
    w_ds = [
        nc.dram_tensor(f"w{ci}", [N, 2 * L], f16, kind="ExternalInput")
        for ci in range(C)
    ]
    # frames, fp16: rows 0:8 c0 top, 8:16 c0 bottom, 16:24 c1 top, 24:32 c1 bot
    # One output tensor per PSUM group: DRAM dependencies are tracked per
    # tensor, so a shared output tensor serializes every output DMA on the
    # previous one's completion semaphore (~1.3us each on the issuing SEQ).
    out_ds = []
    for ci, ck in enumerate(CHUNKS):
        for gi, qs in enumerate(range(0, ck, SUB)):
            ns = min(SUB, ck - qs)
            out_ds.append(
                nc.dram_tensor(
                    f"out{len(out_ds):02d}", [2 * C * HOP, ns], f16,
                    kind="ExternalOutput",
                )
            )

    mw_v = mw_d.ap().rearrange("(b p) k -> p b k", p=128)        # [128, 4, KH]
    mask8_v = mask8_d.ap().rearrange("(b p) k -> p b k", p=128)  # [128, 8, kh8]
    mask16_v = mask16_d.ap().rearrange("(b p) k -> p b k", p=128)
    w_vs = [
        w.ap().rearrange("(b p) l -> p b l", p=128) for w in w_ds
    ]  # [128, 4, 32]

    load = {"DVE": 0.0, "ACT": 0.0, "Pool": 0.0}

    def waterfill(rate, total):
        """Split `total` elements across the engines in `rate` to equalize
        projected finish times given current `load`. Returns {eng: elems}."""
        lo = min(load[e] for e in rate)
        hi = max(load[e] for e in rate) + max(rate.values()) * total + 500
        for _ in range(48):
            mid = (lo + hi) / 2
            got = sum(
                max(0.0, (mid - load[e] - _FIXED[e]) / rate[e]) for e in rate
            )
            if got >= total:
                hi = mid
            else:
                lo = mid
        return {
            e: max(0.0, (hi - load[e] - _FIXED[e]) / rate[e]) for e in rate
        }

    def pick_one(rate, nelem):
        best, bcost = None, None
        for e in rate:
            c = load[e] + rate[e] * nelem + _FIXED[e]
            if bcost is None or c < bcost:
                best, bcost = e, c
        load[best] += rate[best] * nelem + _FIXED[best]
        return best

    with tile.TileContext(nc) as tc:
        with (
            tc.tile_pool(name="const", bufs=1) as cpool,
            tc.tile_pool(name="mwp", bufs=IN_BUFS) as mwp,
            tc.tile_pool(name="maskp", bufs=IN_BUFS) as maskp,
            tc.tile_pool(name="cvp", bufs=3) as cvp,
            tc.tile_pool(name="stp", bufs=ST_BUFS) as stp,
            tc.tile_pool(name="obp", bufs=8) as obp,
            tc.tile_pool(name="pop", bufs=6, space="PSUM") as pop,
        ):
            # Zero-padded stationaries (pre-built on the host): wtp0 cols
            # 0:16 = W' (c0 -> PSUM partitions 0:16), wtp1 cols 16:32 = W'
            # (c1 -> 16:32). The zero columns matmul-write the other
            # channel's partitions with +0, so all 32 partitions are
            # matmul-initialized and accumulate correctly.
            wtps = []
            for ci in range(C):
                wtp = cpool.tile([128, NB, 2 * L], f16, tag=f"wtp{ci}")
                nc.scalar.dma_start(wtp[:], w_vs[ci])
                wtps.append(wtp)

            # PE warmup: harmless matmuls on the weight tile keep the PE
            # marked busy while the first input chunks land, so the real
            # matmuls are costed at the ramped clock.
            if WARMUP_MM:
                pfw = pop.tile([32, SUB], f32, tag="warm", bufs=1)
                wmov = wtps[0][:].rearrange("p b l -> p (b l)")
            for wi in range(WARMUP_MM):
                nc.tensor.matmul(
                    pfw[:, 0 : NB * 2 * L],
                    wtps[0][:, 0, :],
                    wmov,
                    start=(wi == 0),
                    stop=(wi == WARMUP_MM - 1),
                )

            def copy_op(name, out, in_):
                if name == "ACT":
                    nc.scalar.copy(out, in_)
                elif name == "DVE":
                    nc.vector.tensor_copy(out, in_)
                else:
                    nc.gpsimd.tensor_copy(out, in_)

            def mul_op(name, out, in0, in1):
                if name == "DVE":
                    nc.vector.tensor_mul(out, in0, in1)
                else:
                    nc.gpsimd.tensor_mul(out, in0, in1)

            def split_cols(rate, ck, nb=NB):
                """Waterfill nb*ck elements, quantized to whole columns.
                Returns [(engine, col_lo, col_hi)], DVE range first."""
                shares = waterfill(rate, nb * ck)
                cols = {e: int(round(shares[e] / nb)) for e in shares}
                # clip tiny shares, give the remainder to the biggest
                for e in cols:
                    if cols[e] < MIN_COLS:
                        cols[e] = 0
                emax = max(cols, key=lambda e: cols[e])
                cols[emax] += ck - sum(cols.values())
                out, a = [], 0
                order = [e for e in ("DVE", "Pool", "ACT") if e in rate]
                for e in order:
                    n = min(cols.get(e, 0), ck - a)
                    if n > 0:
                        out.append((e, a, a + n))
                        load[e] += rate[e] * n * nb + _FIXED[e]
                        a += n
                if a < ck:  # numeric corner: dump the rest on the first
                    e = out[0][0] if out else "DVE"
                    out.append((e, a, ck))
                    load[e] += rate[e] * (ck - a) * nb + _FIXED[e]
                return out

            off8 = [0]
            off16 = [0]

            def load_phase(ci, k0, ck):
                """DMA in + all conversions for chunk ci (runs one pipeline
                stage ahead of the multiplies so they never wait on it).

                Two modes, chosen per chunk off the load ledger:
                 - mixed: no mask conversion; multiplies run mixed-dtype
                 - conv: ACT converts the chunk's mask to fp16; the DVE
                   multiply band then runs at 2 elem/cycle (spends idle ACT
                   capacity to relieve the DVE)
                """
                is16 = ci in FP16_CHUNKS
                mwq = mwp.tile([128, NB, ck], i8, tag="mwq")
                nc.sync.dma_start(mwq[:], mw_v[:, :, k0 : k0 + ck])
                if is16:
                    mq = maskp.tile([128, C * NB, ck], f16, tag="maskq16", bufs=2)
                    s0 = off16[0]
                    off16[0] += ck
                    mv = mask16_v
                else:
                    mq = maskp.tile([128, C * NB, ck], u8, tag="maskq8")
                    s0 = off8[0]
                    off8[0] += ck
                    mv = mask8_v
                for cc in range(C):
                    nc.sync.dma_start(
                        mq[:, cc * NB : (cc + 1) * NB],
                        mv[:, cc * NB : (cc + 1) * NB, s0 : s0 + ck],
                    )

                ra_w = NB * _CONV_RATE["ACT"]          # mw conv ns/col
                ra_m = NB * C * _CONV_RATE["ACT"]      # mask conv ns/col
                mr_dx = (_F16MUL_RATE if is16 else _MIXED_RATE)["DVE"]
                mr_p = (_F16MUL_RATE if is16 else _MIXED_RATE)["Pool"]
                rd_x = NB * (_CONV_RATE["DVE"] + C * mr_dx)  # DVE band
                rd_m = NB * C * _F16MUL_RATE["DVE"]          # middle-band muls
                rp = NB * C * mr_p                           # Pool band
                best, best_m = (ck, ck), None
                step = max(32, ck // 32)
                ys = [0] if is16 else None
                for x in range(0, ck + 1, step):
                    for y in (range(x, ck + 1, step) if not is16 else [x]):
                        m = max(
                            load["DVE"] + rd_x * x + rd_m * (y - x),
                            load["ACT"] + ra_m * (y - x) + ra_w * (ck - x),
                            load["Pool"] + rp * (ck - y),
                        )
                        if best_m is None or m < best_m:
                            best, best_m = (x, y), m
                x, y = best

                mwf = cvp.tile([128, NB, ck], f16, tag="mwf")
                if x > 0:
                    copy_op("DVE", mwf[:, :, 0:x], mwq[:, :, 0:x])
                    load["DVE"] += _CONV_RATE["DVE"] * NB * x + _FIXED["DVE"]
                if x < ck:
                    copy_op("ACT", mwf[:, :, x:ck], mwq[:, :, x:ck])
                    load["ACT"] += _CONV_RATE["ACT"] * NB * (ck - x) + _FIXED["ACT"]

                mkf = None
                if y > x:
                    mkf = cvp.tile([128, C * NB, ck], f16, tag="mkf", bufs=2)
                    for cc in range(C):
                        copy_op(
                            "ACT",
                            mkf[:, cc * NB : (cc + 1) * NB, x:y],
                            mq[:, cc * NB : (cc + 1) * NB, x:y],
                        )
                        load["ACT"] += _CONV_RATE["ACT"] * NB * (y - x) + _FIXED["ACT"]
                return (mq, mkf, mwf, x, y, is16, ck)

            def mul_phase(state):
                """The st multiplies for a chunk loaded one stage earlier."""
                mq, mkf, mwf, x, y, is16, ck = state
                mr_d = (_F16MUL_RATE if is16 else _MIXED_RATE)["DVE"]
                mr_p = (_F16MUL_RATE if is16 else _MIXED_RATE)["Pool"]
                sts = []
                for cc in range(C):
                    st = stp.tile([128, NB, ck], f16, tag=f"st{cc}")
                    if x > 0:
                        mul_op(
                            "DVE",
                            st[:, :, 0:x],
                            mq[:, cc * NB : (cc + 1) * NB, 0:x],
                            mwf[:, :, 0:x],
                        )
                        load["DVE"] += mr_d * NB * x + _FIXED["DVE"]
                    if y > x:
                        mul_op(
                            "DVE",
                            st[:, :, x:y],
                            mkf[:, cc * NB : (cc + 1) * NB, x:y],
                            mwf[:, :, x:y],
                        )
                        load["DVE"] += (
                            _F16MUL_RATE["DVE"] * NB * (y - x) + _FIXED["DVE"]
                        )
                    if y < ck:
                        mul_op(
                            "Pool",
                            st[:, :, y:ck],
                            mq[:, cc * NB : (cc + 1) * NB, y:ck],
                            mwf[:, :, y:ck],
                        )
                        load["Pool"] += mr_p * NB * (ck - y) + _FIXED["Pool"]
                    sts.append(st)
                return sts

            def back_phase(k0, ck, sts):
                """Matmuls, PSUM->SBUF stage, output DMA, one PSUM group at
                a time."""
                for qs in range(0, ck, SUB):
                    ns = min(SUB, ck - qs)
                    pf = pop.tile([32, SUB], f32, tag="po")
                    for cc in range(C):
                        for ni in range(NB):
                            nc.tensor.matmul(
                                pf[:, 0:ns],
                                wtps[cc][:, ni, :],
                                sts[cc][:, ni, qs : qs + ns],
                                start=(cc == 0 and ni == 0),
                                stop=(cc == C - 1 and ni == NB - 1),
                            )
                    eng = pick_one(_STAGE_RATE, ns)
                    ob = obp.tile([32, SUB], f16, tag="ob")
                    copy_op(eng, ob[:, 0:ns], pf[:, 0:ns])
                    # The DMA is queued one chunk later, when its stage is
                    # long finished, so the issuing SEQ never stalls on it.
                    out_q.append((out_ds[gctr[0]].ap()[:, 0:ns], ob[:, 0:ns]))
                    gctr[0] += 1

            gctr = [0]
            out_q = []  # (dst_ap, src_ap) awaiting their output DMA
            rr = [0]

            def flush_outs():
                engs = [nc.scalar]
                while out_q:
                    dst, srcv = out_q.pop(0)
                    engs[0].dma_start(dst, srcv)
                    rr[0] += 1

            # three-stage software pipeline over chunks:
            #   iteration i: multiplies(i) | matmul/stage/out(i-1) | load(i+1)
            k0s = []
            k0 = 0
            for ck in CHUNKS:
                k0s.append(k0)
                k0 += ck
            loaded = {0: load_phase(0, k0s[0], CHUNKS[0])}
            mulled = {}
            for i in range(len(CHUNKS) + 1):
                if i < len(CHUNKS):
                    mulled[i] = mul_phase(loaded.pop(i))
                if i >= 1:
                    flush_outs()
                    back_phase(k0s[i - 1], CHUNKS[i - 1], mulled.pop(i - 1))
                if i + 1 < len(CHUNKS):
                    loaded[i + 1] = load_phase(i + 1, k0s[i + 1], CHUNKS[i + 1])
            flush_outs()

    nc.compile()
    return nc


def get_nc():
    if "nc" not in _CACHE:
        _CACHE["nc"] = _build_nc()
    return _CACHE["nc"]


_COLS8, _COLS16 = [], []
_k0 = 0
for _ci, _ck in enumerate(CHUNKS):
    (_COLS16 if _ci in FP16_CHUNKS else _COLS8).append((_k0, _k0 + _ck))
    _k0 += _ck


def make_in_maps(mixture_w, est_mask, W):
    mixture_w = np.asarray(mixture_w, np.float32)
    est_mask = np.asarray(est_mask, np.float32)
    W = np.asarray(W, np.float32)
    in_maps = []
    for m in range(M):
        for kh in range(2):
            s0 = kh * KH
            mw = mixture_w[m, :, s0 : s0 + KH]
            s = np.abs(mw).max(axis=1) / 127.0            # [N]
            np.maximum(s, 1e-30, out=s)
            mw_q = np.rint(mw / s[:, None]).astype(np.int8)
            wp = (W.T * (s[:, None] * (2.0**SCALE_BITS / 255.0))).astype(
                np.float16
            )                                             # [N, L]
            wpad = [np.zeros((N, 2 * L), np.float16) for _ in range(C)]
            for c in range(C):
                wpad[c][:, L * c : L * c + L] = wp
            mask = est_mask[m, :, :, s0 : s0 + KH].reshape(C * N, KH)
            m8 = np.concatenate(
                [np.rint(mask[:, a:b] * 255.0) for a, b in _COLS8], axis=1
            ).astype(np.uint8) if _COLS8 else np.zeros((C * N, 0), np.uint8)
            if _COLS16:
                # fp16 mask columns carry the 255 scale so W' stays shared
                m16 = np.concatenate(
                    [mask[:, a:b] * np.float32(255.0) for a, b in _COLS16],
                    axis=1,
                ).astype(np.float16)
            else:
                m16 = np.zeros((C * N, 0), np.float16)
            in_maps.append(
                {
                    "mw": np.ascontiguousarray(mw_q),
                    "mask8": np.ascontiguousarray(m8),
                    "mask16": np.ascontiguousarray(m16),
                    "w0": wpad[0],
                    "w1": wpad[1],
                }
            )
    return in_maps


def stitch(results):
    """results: 8 per-core dicts of per-group frame arrays, (m, kh) order."""
    inv = np.float32(2.0**-SCALE_BITS)
    out = np.zeros((M, C, T), np.float32)
    for m in range(M):
        for kh in range(2):
            r = results[2 * m + kh]
            fr = np.concatenate(
                [r[k] for k in sorted(r) if k.startswith("out")], axis=1
            ).astype(np.float32) * inv                         # [32, KH]
            half = np.zeros((C, HOP, QH), np.float32)
            for c in range(C):
                top = fr[16 * c : 16 * c + 8]       # frames[r, j]
                bot = fr[16 * c + 8 : 16 * c + 16]  # frames[r+8, j]
                half[c, :, :KH] = top
                half[c, :, 1:] += bot
            # [C, HOP, QH] -> [C, TH] with t = 8q + r
            half_t = half.transpose(0, 2, 1).reshape(C, TH)
            if kh == 0:
                out[m, :, :TH] = half_t
            else:
                out[m, :, KH * HOP :] += half_t
    return out


def kernel(mixture_w, est_mask, W):
    from concourse.bass_utils import run_bass_kernel_spmd

    nc = get_nc()
    in_maps = make_in_maps(mixture_w, est_mask, W)
    res = run_bass_kernel_spmd(nc, in_maps, list(range(M * 2)))
    return stitch(list(res.results))
